# revision 17
# baseline (speedup 1.0000x reference)
"""Bilinear decoder kernel for Trainium2 (8 NeuronCores, SPMD).

Computes score[b] = head[b]^T @ relation_matrices[relation_ids[b]] @ tail[b]
for b in [0, 4096).

Strategy (relation-grouped subtiles, deduped matrices, per-subtile DVE):
  Host: chunk each relation's samples into segments of <=256 rows. Segments
  with >128 rows are "pairs" (one full 128-row subtile + a remainder
  subtile of <=cap rows, sharing one matrix); <=128-row segments are
  "singles". One SPMD program with G2 pair slots + G1 single slots
  (light cores get zero-filled slots; program uniform across cores).
  Bundles (bf16):
    pair [128, 1280]: mat 512 | h0 256 | t0 256 | t1 256; the bundle at
        index rh (p1, or p0 if only one pair) carries a prefix
        [G2*2*cap] with ALL pairs' remainder heads compacted to cap
        cols per contraction chunk.
    sg  [128, G1*1024]: per single: mat | h | t
  DMA (HWDGE FIFO per ring): sync [p0, p2, ...], scalar [p1, sg]. The
  first two pair bundles stream concurrently on the two rings and land
  first; the remainder heads ride p1 (still well before the first
  remainder matmul needs them) so p0 stays small; the singles bundle
  (needed last) queues behind p1.
  Device per pair: 4 accumulated bf16 matmuls into one [128,512] psum
  (full subtile -> cols [0,256); remainder subtile -> cols [256,512),
  partitions [0,cap) only), then ONE fused DVE tensor_tensor [128,512]
  (psum * t0|t1 -> bf16 scr; t0/t1 are adjacent in the bundle) and two
  per-subtile reduce_sums -> out_sb columns. Rows >= nb of a remainder
  subtile may hold NaN (uninitialized psum * zero tail); row-wise
  reduction confines them to out rows the host never reads. Singles are
  computed last: their short DVE ops form the tail before the single
  output DMA (sync ring). Host: scatter scores back per slot.
  Measured: mean exec ~20.8-21.4us, max-core 21.0-22.1us (baseline
  21.6us); within-run core spread ~100-600ns.

bf16 inputs, fp32 PSUM accumulate + fp32 reduce: absmax-relative error
3.0e-3 on the full problem (gate 2e-2).

HW-measured notes (NTFF traces; exec window = [first MEMSET .. last
instruction], includes a fixed ~8.45us NEFF teardown sweep of 253
semaphore clears + ~0.75us lead-in):
  - tensor_tensor_reduce with PSUM in0 crashes the DVE
    (NRT_EXEC_UNIT_UNRECOVERABLE) in both real-out and broadcast-dummy
    forms; qr.py's working TTR reads SBUF only. TT+reduce it is.
  - ANY use of the Scalar engine's activation path (even one warmup op)
    downclocks the whole chip ~20% (teardown 8.45us -> 10.1us, matmuls
    394 -> 474ns): power/clock-domain sharing. Never use ACT compute.
    This also explains the prior session's unexplained ACT-reduce loss.
  - A small separate remainder-heads DMA mid-FIFO adds ~1us of
    inter-DMA bubbles and starves the DVE chain; ride it on a pair
    bundle instead.
  - HWDGE: issue ~0.65us + DGE delay ~0.65us + ~0.9us completion
    receipt; the two rings share ~350-400GB/s of HBM stream.
"""

import numpy as np
import ml_dtypes

P = 128
DIM = 256
NCORES = 8
BF = ml_dtypes.bfloat16

_prog_cache = {}

# test-harness knobs: set TRACE=True before calling kernel() to capture an
# NTFF profile; the BassKernelResults lands in LAST_RESULT.
TRACE = False
LAST_RESULT = None

PW = 5 * DIM  # pair payload cols: mat 512 | h0 256 | t0 256 | t1 256
SW = 4 * DIM  # single cols: mat 512 | h 256 | t 256


def _build(G2, G1, cap):
    import concourse.bacc as bacc
    import concourse.mybir as mybir
    import concourse.tile as tile

    f32 = mybir.dt.float32
    bf16 = mybir.dt.bfloat16
    SUBS = G1 + 2 * G2
    RB = G2 * 2 * cap  # remainder-heads block cols (prefix of p0)

    nc = bacc.Bacc("TRN2", target_bir_lowering=False)
    rh = 1 if G2 > 1 else 0  # which pair bundle carries the remh prefix
    pair_in = [
        nc.dram_tensor(
            f"p{g}", [P, (RB if g == rh else 0) + PW], bf16, kind="ExternalInput"
        )
        for g in range(G2)
    ]
    sg_in = (
        nc.dram_tensor("sg", [P, G1 * SW], bf16, kind="ExternalInput") if G1 else None
    )
    out = nc.dram_tensor("out", [P, SUBS], f32, kind="ExternalOutput")

    with tile.TileContext(nc) as tc:
        with (
            tc.tile_pool(name="blk", bufs=G2 + 1) as blk_pool,
            tc.tile_pool(name="scr", bufs=2) as scr_pool,
            tc.tile_pool(name="o", bufs=1) as o_pool,
            tc.tile_pool(name="psum", bufs=2, space="PSUM") as psum_pool,
        ):
            ptiles = []
            for g in range(G2):
                t = blk_pool.tile([P, (RB if g == rh else 0) + PW], bf16, tag="blk")
                eng = nc.sync if g % 2 == 0 else nc.scalar
                eng.dma_start(out=t[:], in_=pair_in[g][:, :])
                ptiles.append(t)
            sg_tile = None
            if G1:
                sg_tile = blk_pool.tile([P, G1 * SW], bf16, tag="blk")
                eng = nc.sync if G2 % 2 == 0 else nc.scalar
                eng.dma_start(out=sg_tile[:], in_=sg_in[:, :])

            out_sb = o_pool.tile([P, SUBS], f32)

            for g in range(G2):
                tl = ptiles[g]
                off = RB if g == rh else 0
                col = 2 * g
                if g == 0:
                    # pair 0 runs per-subtile (separate psum tiles) so the
                    # DVE chain starts after just 2 matmuls instead of 4 —
                    # a [128,512] fused TT here would wait on the whole
                    # pair (tile-granular psum dependency).
                    ps0 = psum_pool.tile([P, DIM], f32, tag="ps0")
                    for c in range(2):
                        nc.tensor.matmul(
                            out=ps0[:],
                            lhsT=tl[:, off + 2 * DIM + c * P : off + 2 * DIM + (c + 1) * P],
                            rhs=tl[:, off + c * DIM : off + (c + 1) * DIM],
                            start=(c == 0),
                            stop=(c == 1),
                        )
                    scr0 = scr_pool.tile([P, DIM], bf16, tag="scr")
                    nc.vector.tensor_tensor(
                        out=scr0[:],
                        in0=ps0[:],
                        in1=tl[:, off + 3 * DIM : off + 4 * DIM],
                        op=mybir.AluOpType.mult,
                    )
                    nc.vector.reduce_sum(
                        out=out_sb[:, col : col + 1],
                        in_=scr0[:],
                        axis=mybir.AxisListType.X,
                    )
                    ps1 = psum_pool.tile([P, DIM], f32, tag="ps0")
                    for c in range(2):
                        nc.tensor.matmul(
                            out=ps1[0:cap, :],
                            lhsT=ptiles[rh][
                                :, g * 2 * cap + c * cap : g * 2 * cap + (c + 1) * cap
                            ],
                            rhs=tl[:, off + c * DIM : off + (c + 1) * DIM],
                            start=(c == 0),
                            stop=(c == 1),
                        )
                    scr1 = scr_pool.tile([P, DIM], bf16, tag="scr")
                    nc.vector.tensor_tensor(
                        out=scr1[0:cap, :],
                        in0=ps1[0:cap, :],
                        in1=tl[0:cap, off + 4 * DIM : off + 5 * DIM],
                        op=mybir.AluOpType.mult,
                    )
                    nc.vector.reduce_sum(
                        out=out_sb[0:cap, col + 1 : col + 2],
                        in_=scr1[0:cap, :],
                        axis=mybir.AxisListType.X,
                    )
                    continue
                ps = psum_pool.tile([P, 2 * DIM], f32, tag="psp")
                # full subtile -> psum cols [0,256)
                for c in range(2):
                    nc.tensor.matmul(
                        out=ps[:, 0:DIM],
                        lhsT=tl[:, off + 2 * DIM + c * P : off + 2 * DIM + (c + 1) * P],
                        rhs=tl[:, off + c * DIM : off + (c + 1) * DIM],
                        start=(c == 0),
                        stop=(c == 1),
                    )
                # remainder subtile -> psum cols [256,512), partitions [0,cap)
                for c in range(2):
                    nc.tensor.matmul(
                        out=ps[0:cap, DIM : 2 * DIM],
                        lhsT=ptiles[rh][
                            :, g * 2 * cap + c * cap : g * 2 * cap + (c + 1) * cap
                        ],
                        rhs=tl[:, off + c * DIM : off + (c + 1) * DIM],
                        start=(c == 0),
                        stop=(c == 1),
                    )
                # one fused multiply over both subtiles (t0|t1 adjacent)
                scr = scr_pool.tile([P, 2 * DIM], bf16, tag="scr")
                nc.vector.tensor_tensor(
                    out=scr[:],
                    in0=ps[:],
                    in1=tl[:, off + 3 * DIM : off + 5 * DIM],
                    op=mybir.AluOpType.mult,
                )
                nc.vector.reduce_sum(
                    out=out_sb[:, col : col + 1],
                    in_=scr[:, 0:DIM],
                    axis=mybir.AxisListType.X,
                )
                nc.vector.reduce_sum(
                    out=out_sb[0:cap, col + 1 : col + 2],
                    in_=scr[0:cap, DIM : 2 * DIM],
                    axis=mybir.AxisListType.X,
                )

            for q in range(G1):
                base = q * SW
                ps = psum_pool.tile([P, DIM], f32, tag="pss")
                for c in range(2):
                    nc.tensor.matmul(
                        out=ps[:],
                        lhsT=sg_tile[
                            :, base + 2 * DIM + c * P : base + 2 * DIM + (c + 1) * P
                        ],
                        rhs=sg_tile[:, base + c * DIM : base + (c + 1) * DIM],
                        start=(c == 0),
                        stop=(c == 1),
                    )
                scr = scr_pool.tile([P, DIM], bf16, tag="scr")
                nc.vector.tensor_tensor(
                    out=scr[:],
                    in0=ps[:],
                    in1=sg_tile[:, base + 3 * DIM : base + 4 * DIM],
                    op=mybir.AluOpType.mult,
                )
                nc.vector.reduce_sum(
                    out=out_sb[:, 2 * G2 + q : 2 * G2 + q + 1],
                    in_=scr[:],
                    axis=mybir.AxisListType.X,
                )

            nc.sync.dma_start(out=out[:, :], in_=out_sb[:])

    nc.compile()
    return nc


def _plan(ids, R):
    """Chunk each relation into <=256-sample segments: pairs (>128 rows)
    and singles (<=128). Round-robin to cores, padded with None to uniform
    (G2, G1). cap = max remainder rows, rounded up to a multiple of 8."""
    pairs, singles = [], []
    for r in range(R):
        idxs = np.nonzero(ids == r)[0]
        for s in range(0, len(idxs), 2 * P):
            seg = idxs[s : s + 2 * P]
            (pairs if len(seg) > P else singles).append((r, seg))
    if not pairs and not singles:
        singles.append((0, np.empty(0, np.int64)))
    G2 = -(-len(pairs) // NCORES) if pairs else 0
    G1 = -(-len(singles) // NCORES) if singles else 0
    cap = 8
    for r, seg in pairs:
        cap = max(cap, len(seg) - P)
    cap = (cap + 7) // 8 * 8
    cores = []
    for k in range(NCORES):
        pk = pairs[k * G2 : (k + 1) * G2] if G2 else []
        sk = singles[k * G1 : (k + 1) * G1] if G1 else []
        pk += [None] * (G2 - len(pk))
        sk += [None] * (G1 - len(sk))
        cores.append((pk, sk))
    return G2, G1, cap, cores


def _core_inputs(head, tail, mstack, pk, sk, G2, G1, cap):
    inp = {}
    RB = G2 * 2 * cap
    rh = 1 if G2 > 1 else 0
    for g in range(G2):
        off = RB if g == rh else 0
        blk = np.zeros((P, off + PW), BF)
        if pk[g] is not None:
            r, seg = pk[g]
            blk[:, off : off + 2 * DIM] = mstack[r]
            s0, s1 = seg[:P], seg[P:]
            ht = head[s0].astype(BF).T  # [DIM, 128]
            blk[:, off + 2 * DIM : off + 2 * DIM + P] = ht[:P, :]
            blk[:, off + 2 * DIM + P : off + 3 * DIM] = ht[P:, :]
            blk[:, off + 3 * DIM : off + 4 * DIM] = tail[s0].astype(BF)
            nb = len(s1)
            blk[:nb, off + 4 * DIM : off + 5 * DIM] = tail[s1].astype(BF)
        inp[f"p{g}"] = blk
    if G2:
        # remainder heads ride p{rh}'s prefix
        p0 = inp[f"p{rh}"]
        for g in range(G2):
            if pk[g] is None:
                continue
            r, seg = pk[g]
            s1 = seg[P:]
            nb = len(s1)
            if nb:
                ht1 = head[s1].astype(BF).T  # [DIM, nb]
                p0[:, g * 2 * cap : g * 2 * cap + nb] = ht1[:P, :]
                p0[:, g * 2 * cap + cap : g * 2 * cap + cap + nb] = ht1[P:, :]
    if G1:
        sg = np.zeros((P, G1 * SW), BF)
        for q in range(G1):
            if sk[q] is not None:
                r, seg = sk[q]
                base = q * SW
                sg[:, base : base + 2 * DIM] = mstack[r]
                nb = len(seg)
                ht = head[seg].astype(BF).T
                sg[:, base + 2 * DIM : base + 2 * DIM + nb] = ht[:P, :]
                sg[:, base + 2 * DIM + P : base + 2 * DIM + P + nb] = ht[P:, :]
                sg[:nb, base + 3 * DIM : base + 4 * DIM] = tail[seg].astype(BF)
        inp["sg"] = sg
    return inp


def kernel(head, relation_ids, tail, relation_matrices):
    head = np.ascontiguousarray(np.asarray(head), dtype=np.float32)
    tail = np.ascontiguousarray(np.asarray(tail), dtype=np.float32)
    mats = np.ascontiguousarray(np.asarray(relation_matrices), dtype=np.float32)
    ids = np.asarray(relation_ids).astype(np.int64)
    B, D = head.shape
    R = mats.shape[0]
    assert D == DIM

    G2, G1, cap, cores = _plan(ids, R)
    # [R, P, 2*DIM] bf16: mstack[r, p, c*256+j] = M_r[c*128+p, j]
    mstack = np.ascontiguousarray(
        mats.reshape(R, 2, P, DIM).transpose(0, 2, 1, 3).reshape(R, P, 2 * DIM)
    ).astype(BF)

    in_maps = [
        _core_inputs(head, tail, mstack, pk, sk, G2, G1, cap) for pk, sk in cores
    ]

    key = (G2, G1, cap)
    if key not in _prog_cache:
        _prog_cache[key] = _build(G2, G1, cap)
    nc = _prog_cache[key]

    from concourse.bass_utils import run_bass_kernel_spmd

    kwargs = {}
    if TRACE:
        kwargs = dict(trace=True, trace_cores=list(range(NCORES)))
    try:
        res = run_bass_kernel_spmd(
            nc, in_maps, core_ids=list(range(NCORES)), **kwargs
        )
    except Exception:
        # a previous crashed session can leave the device wedged; one retry
        # after the error has been consumed usually succeeds
        import time as _time

        _time.sleep(2)
        res = run_bass_kernel_spmd(
            nc, in_maps, core_ids=list(range(NCORES)), **kwargs
        )
    global LAST_RESULT
    LAST_RESULT = res

    scores = np.zeros(B, np.float32)
    for k in range(NCORES):
        o = res.results[k]["out"]  # [P, SUBS]
        pk, sk = cores[k]
        for g in range(G2):
            if pk[g] is None:
                continue
            r, seg = pk[g]
            s0, s1 = seg[:P], seg[P:]
            scores[s0] = o[:P, 2 * g]
            scores[s1] = o[: len(s1), 2 * g + 1]
        for q in range(G1):
            if sk[q] is None:
                continue
            r, seg = sk[q]
            scores[seg] = o[: len(seg), 2 * G2 + q]
    return scores


# revision 18
# speedup vs baseline: 1.0655x; 1.0655x over previous
"""Bilinear decoder kernel for Trainium2 (8 NeuronCores, SPMD).

Computes score[b] = head[b]^T @ relation_matrices[relation_ids[b]] @ tail[b]
for b in [0, 4096).

Strategy (relation-grouped subtiles, deduped matrices, per-subtile DVE):
  Host: chunk each relation's samples into segments of <=256 rows. Segments
  with >128 rows are "pairs" (one full 128-row subtile + a remainder
  subtile of <=cap rows, sharing one matrix); <=128-row segments are
  "singles". One SPMD program with G2 pair slots + G1 single slots
  (light cores get zero-filled slots; program uniform across cores).
  Bundles (bf16):
    pair [128, 1280]: mat 512 | h0 256 | t0 256 | t1 256; the bundle at
        index rh (p1, or p0 if only one pair) carries a prefix
        [G2*2*cap] with ALL pairs' remainder heads compacted to cap
        cols per contraction chunk.
    sg  [128, G1*1024]: per single: mat | h | t
  DMA (HWDGE FIFO per ring): sync [p0, p2, ...], scalar [p1, sg]. The
  first two pair bundles stream concurrently on the two rings and land
  first; the remainder heads ride p1 (still well before the first
  remainder matmul needs them) so p0 stays small; the singles bundle
  (needed last) queues behind p1.
  Device per pair: 4 accumulated bf16 matmuls into one [128,512] psum
  (full subtile -> cols [0,256); remainder subtile -> cols [256,512),
  partitions [0,cap) only), then ONE fused DVE tensor_tensor [128,512]
  (psum * t0|t1 -> bf16 scr; t0/t1 are adjacent in the bundle) and two
  per-subtile reduce_sums -> out_sb columns. Rows >= nb of a remainder
  subtile may hold NaN (uninitialized psum * zero tail); row-wise
  reduction confines them to out rows the host never reads. Singles are
  computed last: their short DVE ops form the tail before the single
  output DMA (sync ring). Host: scatter scores back per slot.
  Measured: mean exec ~20.8-21.4us, max-core 21.0-22.1us (baseline
  21.6us); within-run core spread ~100-600ns.

bf16 inputs, fp32 PSUM accumulate + fp32 reduce: absmax-relative error
3.0e-3 on the full problem (gate 2e-2).

HW-measured notes (NTFF traces; exec window = [first MEMSET .. last
instruction], includes a fixed ~8.45us NEFF teardown sweep of 253
semaphore clears + ~0.75us lead-in):
  - tensor_tensor_reduce with PSUM in0 crashes the DVE
    (NRT_EXEC_UNIT_UNRECOVERABLE) in both real-out and broadcast-dummy
    forms; qr.py's working TTR reads SBUF only. TT+reduce it is.
  - ANY use of the Scalar engine's activation path (even one warmup op)
    downclocks the whole chip ~20% (teardown 8.45us -> 10.1us, matmuls
    394 -> 474ns): power/clock-domain sharing. Never use ACT compute.
    This also explains the prior session's unexplained ACT-reduce loss.
  - A small separate remainder-heads DMA mid-FIFO adds ~1us of
    inter-DMA bubbles and starves the DVE chain; ride it on a pair
    bundle instead.
  - HWDGE: issue ~0.65us + DGE delay ~0.65us + ~0.9us completion
    receipt; the two rings share ~350-400GB/s of HBM stream.
"""

import numpy as np
import ml_dtypes

P = 128
DIM = 256
NCORES = 8
BF = ml_dtypes.bfloat16

_prog_cache = {}

# test-harness knobs: set TRACE=True before calling kernel() to capture an
# NTFF profile; the BassKernelResults lands in LAST_RESULT.
TRACE = False
LAST_RESULT = None

PW = 5 * DIM  # pair payload cols: mat 512 | h0 256 | t0 256 | t1 256
SW = 4 * DIM  # single cols: mat 512 | h 256 | t 256


def _build(G2, G1, cap):
    import concourse.bacc as bacc
    import concourse.mybir as mybir
    import concourse.tile as tile

    f32 = mybir.dt.float32
    bf16 = mybir.dt.bfloat16
    SUBS = G1 + 2 * G2
    RB = G2 * 2 * cap  # remainder-heads block cols (prefix of p0)

    nc = bacc.Bacc("TRN2", target_bir_lowering=False)
    rh = 1 if G2 > 1 else 0  # which pair bundle carries the remh prefix
    pair_in = [
        nc.dram_tensor(
            f"p{g}", [P, (RB if g == rh else 0) + PW], bf16, kind="ExternalInput"
        )
        for g in range(G2)
    ]
    sg_in = (
        nc.dram_tensor("sg", [P, G1 * SW], bf16, kind="ExternalInput") if G1 else None
    )
    out = nc.dram_tensor("out", [P, SUBS], f32, kind="ExternalOutput")

    with tile.TileContext(nc) as tc:
        with (
            tc.tile_pool(name="blk", bufs=G2 + 1) as blk_pool,
            tc.tile_pool(name="scr", bufs=2) as scr_pool,
            tc.tile_pool(name="o", bufs=1) as o_pool,
            tc.tile_pool(name="psum", bufs=4, space="PSUM") as psum_pool,
        ):
            ptiles = []
            for g in range(G2):
                t = blk_pool.tile([P, (RB if g == rh else 0) + PW], bf16, tag="blk")
                eng = nc.sync if g % 2 == 0 else nc.scalar
                eng.dma_start(out=t[:], in_=pair_in[g][:, :])
                ptiles.append(t)
            sg_tile = None
            if G1:
                sg_tile = blk_pool.tile([P, G1 * SW], bf16, tag="blk")
                eng = nc.sync if G2 % 2 == 0 else nc.scalar
                eng.dma_start(out=sg_tile[:], in_=sg_in[:, :])

            out_sb = o_pool.tile([P, SUBS], f32)

            for g in range(G2):
                tl = ptiles[g]
                off = RB if g == rh else 0
                col = 2 * g
                ps = psum_pool.tile([P, 2 * DIM], f32, tag="psp")
                # full subtile -> psum cols [0,256)
                for c in range(2):
                    nc.tensor.matmul(
                        out=ps[:, 0:DIM],
                        lhsT=tl[:, off + 2 * DIM + c * P : off + 2 * DIM + (c + 1) * P],
                        rhs=tl[:, off + c * DIM : off + (c + 1) * DIM],
                        start=(c == 0),
                        stop=(c == 1),
                    )
                # remainder subtile -> psum cols [256,512), partitions [0,cap)
                for c in range(2):
                    nc.tensor.matmul(
                        out=ps[0:cap, DIM : 2 * DIM],
                        lhsT=ptiles[rh][
                            :, g * 2 * cap + c * cap : g * 2 * cap + (c + 1) * cap
                        ],
                        rhs=tl[:, off + c * DIM : off + (c + 1) * DIM],
                        start=(c == 0),
                        stop=(c == 1),
                    )
                # one fused multiply over both subtiles (t0|t1 adjacent)
                scr = scr_pool.tile([P, 2 * DIM], bf16, tag="scr")
                nc.vector.tensor_tensor(
                    out=scr[:],
                    in0=ps[:],
                    in1=tl[:, off + 3 * DIM : off + 5 * DIM],
                    op=mybir.AluOpType.mult,
                )
                nc.vector.reduce_sum(
                    out=out_sb[:, col : col + 1],
                    in_=scr[:, 0:DIM],
                    axis=mybir.AxisListType.X,
                )
                nc.vector.reduce_sum(
                    out=out_sb[0:cap, col + 1 : col + 2],
                    in_=scr[0:cap, DIM : 2 * DIM],
                    axis=mybir.AxisListType.X,
                )

            for q in range(G1):
                base = q * SW
                ps = psum_pool.tile([P, DIM], f32, tag="pss")
                for c in range(2):
                    nc.tensor.matmul(
                        out=ps[:],
                        lhsT=sg_tile[
                            :, base + 2 * DIM + c * P : base + 2 * DIM + (c + 1) * P
                        ],
                        rhs=sg_tile[:, base + c * DIM : base + (c + 1) * DIM],
                        start=(c == 0),
                        stop=(c == 1),
                    )
                scr = scr_pool.tile([P, DIM], bf16, tag="scr")
                nc.vector.tensor_tensor(
                    out=scr[:],
                    in0=ps[:],
                    in1=sg_tile[:, base + 3 * DIM : base + 4 * DIM],
                    op=mybir.AluOpType.mult,
                )
                nc.vector.reduce_sum(
                    out=out_sb[:, 2 * G2 + q : 2 * G2 + q + 1],
                    in_=scr[:],
                    axis=mybir.AxisListType.X,
                )

            nc.sync.dma_start(out=out[:, :], in_=out_sb[:])

    nc.compile()
    return nc


def _plan(ids, R):
    """Chunk each relation into <=256-sample segments: pairs (>128 rows)
    and singles (<=128). Round-robin to cores, padded with None to uniform
    (G2, G1). cap = max remainder rows, rounded up to a multiple of 8."""
    pairs, singles = [], []
    for r in range(R):
        idxs = np.nonzero(ids == r)[0]
        for s in range(0, len(idxs), 2 * P):
            seg = idxs[s : s + 2 * P]
            (pairs if len(seg) > P else singles).append((r, seg))
    if not pairs and not singles:
        singles.append((0, np.empty(0, np.int64)))
    G2 = -(-len(pairs) // NCORES) if pairs else 0
    G1 = -(-len(singles) // NCORES) if singles else 0
    cap = 8
    for r, seg in pairs:
        cap = max(cap, len(seg) - P)
    cap = (cap + 7) // 8 * 8
    cores = []
    for k in range(NCORES):
        pk = pairs[k * G2 : (k + 1) * G2] if G2 else []
        sk = singles[k * G1 : (k + 1) * G1] if G1 else []
        pk += [None] * (G2 - len(pk))
        sk += [None] * (G1 - len(sk))
        cores.append((pk, sk))
    return G2, G1, cap, cores


def _core_inputs(head, tail, mstack, pk, sk, G2, G1, cap):
    inp = {}
    RB = G2 * 2 * cap
    rh = 1 if G2 > 1 else 0
    for g in range(G2):
        off = RB if g == rh else 0
        blk = np.zeros((P, off + PW), BF)
        if pk[g] is not None:
            r, seg = pk[g]
            blk[:, off : off + 2 * DIM] = mstack[r]
            s0, s1 = seg[:P], seg[P:]
            ht = head[s0].astype(BF).T  # [DIM, 128]
            blk[:, off + 2 * DIM : off + 2 * DIM + P] = ht[:P, :]
            blk[:, off + 2 * DIM + P : off + 3 * DIM] = ht[P:, :]
            blk[:, off + 3 * DIM : off + 4 * DIM] = tail[s0].astype(BF)
            nb = len(s1)
            blk[:nb, off + 4 * DIM : off + 5 * DIM] = tail[s1].astype(BF)
        inp[f"p{g}"] = blk
    if G2:
        # remainder heads ride p{rh}'s prefix
        p0 = inp[f"p{rh}"]
        for g in range(G2):
            if pk[g] is None:
                continue
            r, seg = pk[g]
            s1 = seg[P:]
            nb = len(s1)
            if nb:
                ht1 = head[s1].astype(BF).T  # [DIM, nb]
                p0[:, g * 2 * cap : g * 2 * cap + nb] = ht1[:P, :]
                p0[:, g * 2 * cap + cap : g * 2 * cap + cap + nb] = ht1[P:, :]
    if G1:
        sg = np.zeros((P, G1 * SW), BF)
        for q in range(G1):
            if sk[q] is not None:
                r, seg = sk[q]
                base = q * SW
                sg[:, base : base + 2 * DIM] = mstack[r]
                nb = len(seg)
                ht = head[seg].astype(BF).T
                sg[:, base + 2 * DIM : base + 2 * DIM + nb] = ht[:P, :]
                sg[:, base + 2 * DIM + P : base + 2 * DIM + P + nb] = ht[P:, :]
                sg[:nb, base + 3 * DIM : base + 4 * DIM] = tail[seg].astype(BF)
        inp["sg"] = sg
    return inp


def kernel(head, relation_ids, tail, relation_matrices):
    head = np.ascontiguousarray(np.asarray(head), dtype=np.float32)
    tail = np.ascontiguousarray(np.asarray(tail), dtype=np.float32)
    mats = np.ascontiguousarray(np.asarray(relation_matrices), dtype=np.float32)
    ids = np.asarray(relation_ids).astype(np.int64)
    B, D = head.shape
    R = mats.shape[0]
    assert D == DIM

    G2, G1, cap, cores = _plan(ids, R)
    # [R, P, 2*DIM] bf16: mstack[r, p, c*256+j] = M_r[c*128+p, j]
    mstack = np.ascontiguousarray(
        mats.reshape(R, 2, P, DIM).transpose(0, 2, 1, 3).reshape(R, P, 2 * DIM)
    ).astype(BF)

    in_maps = [
        _core_inputs(head, tail, mstack, pk, sk, G2, G1, cap) for pk, sk in cores
    ]

    key = (G2, G1, cap)
    if key not in _prog_cache:
        _prog_cache[key] = _build(G2, G1, cap)
    nc = _prog_cache[key]

    from concourse.bass_utils import run_bass_kernel_spmd

    kwargs = {}
    if TRACE:
        kwargs = dict(trace=True, trace_cores=list(range(NCORES)))
    try:
        res = run_bass_kernel_spmd(
            nc, in_maps, core_ids=list(range(NCORES)), **kwargs
        )
    except Exception:
        # a previous crashed session can leave the device wedged; one retry
        # after the error has been consumed usually succeeds
        import time as _time

        _time.sleep(2)
        res = run_bass_kernel_spmd(
            nc, in_maps, core_ids=list(range(NCORES)), **kwargs
        )
    global LAST_RESULT
    LAST_RESULT = res

    scores = np.zeros(B, np.float32)
    for k in range(NCORES):
        o = res.results[k]["out"]  # [P, SUBS]
        pk, sk = cores[k]
        for g in range(G2):
            if pk[g] is None:
                continue
            r, seg = pk[g]
            s0, s1 = seg[:P], seg[P:]
            scores[s0] = o[:P, 2 * g]
            scores[s1] = o[: len(s1), 2 * g + 1]
        for q in range(G1):
            if sk[q] is None:
                continue
            r, seg = sk[q]
            scores[seg] = o[: len(seg), 2 * G2 + q]
    return scores


# revision 22
# speedup vs baseline: 1.1086x; 1.0405x over previous
"""Bilinear decoder kernel for Trainium2 (8 NeuronCores, SPMD).

Computes score[b] = head[b]^T @ relation_matrices[relation_ids[b]] @ tail[b]
for b in [0, 4096).

Strategy (relation-grouped subtiles, deduped matrices, per-subtile DVE):
  Host: chunk each relation's samples into segments of <=256 rows. Segments
  with >128 rows are "pairs" (one full 128-row subtile + a remainder
  subtile of <=cap rows, sharing one matrix); <=128-row segments are
  "singles". One SPMD program with G2 pair slots + G1 single slots
  (light cores get zero-filled slots; program uniform across cores).
  Bundles (bf16):
    pair [128, 1280]: mat 512 | h0 256 | t0 256 | t1 256; the bundle at
        index rh (p1, or p0 if only one pair) carries a prefix
        [G2*2*cap] with ALL pairs' remainder heads compacted to cap
        cols per contraction chunk.
    sg  [128, G1*1024]: per single: mat | h | t
  DMA (HWDGE FIFO per ring): sync [p0, p2, ...], scalar [p1, sg]. The
  first two pair bundles stream concurrently on the two rings and land
  first; the remainder heads ride p1 (still well before the first
  remainder matmul needs them) so p0 stays small; the singles bundle
  (needed last) queues behind p1.
  Device per pair: 4 accumulated bf16 matmuls into one [128,512] psum
  (full subtile -> cols [0,256); remainder subtile -> cols [256,512),
  partitions [0,cap) only), then ONE fused DVE tensor_tensor [128,512]
  (psum * t0|t1 -> bf16 scr; t0/t1 are adjacent in the bundle) and two
  per-subtile reduce_sums -> out_sb columns. Rows >= nb of a remainder
  subtile may hold NaN (uninitialized psum * zero tail); row-wise
  reduction confines them to out rows the host never reads. Singles are
  computed last: their short DVE ops form the tail before the single
  output DMA (sync ring). Host: scatter scores back per slot.
  Measured: mean exec ~20.8-21.4us, max-core 21.0-22.1us (baseline
  21.6us); within-run core spread ~100-600ns.

bf16 inputs, fp32 PSUM accumulate + fp32 reduce: absmax-relative error
3.0e-3 on the full problem (gate 2e-2).

HW-measured notes (NTFF traces; exec window = [first MEMSET .. last
instruction], includes a fixed ~8.45us NEFF teardown sweep of 253
semaphore clears + ~0.75us lead-in):
  - tensor_tensor_reduce with PSUM in0 crashes the DVE
    (NRT_EXEC_UNIT_UNRECOVERABLE) in both real-out and broadcast-dummy
    forms; qr.py's working TTR reads SBUF only. TT+reduce it is.
  - ANY use of the Scalar engine's activation path (even one warmup op)
    downclocks the whole chip ~20% (teardown 8.45us -> 10.1us, matmuls
    394 -> 474ns): power/clock-domain sharing. Never use ACT compute.
    This also explains the prior session's unexplained ACT-reduce loss.
  - A small separate remainder-heads DMA mid-FIFO adds ~1us of
    inter-DMA bubbles and starves the DVE chain; ride it on a pair
    bundle instead.
  - HWDGE: issue ~0.65us + DGE delay ~0.65us + ~0.9us completion
    receipt; the two rings share ~350-400GB/s of HBM stream.
"""

import numpy as np
import ml_dtypes

P = 128
DIM = 256
NCORES = 8
BF = ml_dtypes.bfloat16

_prog_cache = {}

# test-harness knobs: set TRACE=True before calling kernel() to capture an
# NTFF profile; the BassKernelResults lands in LAST_RESULT.
TRACE = False
LAST_RESULT = None

PW = 5 * DIM  # pair payload cols: mat 512 | h0 256 | t0 256 | t1 256
SW = 4 * DIM  # single cols: mat 512 | h 256 | t 256


def _build(G2, G1, cap):
    import concourse.bacc as bacc
    import concourse.mybir as mybir
    import concourse.tile as tile

    f32 = mybir.dt.float32
    bf16 = mybir.dt.bfloat16
    SUBS = G1 + 2 * G2
    RB = G2 * 2 * cap  # remainder-heads block cols (prefix of p0)

    nc = bacc.Bacc("TRN2", target_bir_lowering=False)
    rh = 1 if G2 > 1 else 0  # which pair bundle carries the remh prefix
    # when G2>1, pair 0's t1 also rides p[rh]'s prefix (after remh) so p0
    # shrinks to [mat|h0|t0] (256KB) and lands ~0.4us earlier; pair 0 then
    # runs per-subtile DVE so the chain starts after just 2 matmuls.
    t1x = G2 > 1

    def _w(g):
        w = PW
        if g == rh:
            w += RB + (DIM if t1x else 0)
        if t1x and g == 0:
            w -= DIM
        return w

    pair_in = [
        nc.dram_tensor(f"p{g}", [P, _w(g)], bf16, kind="ExternalInput")
        for g in range(G2)
    ]
    sg_in = (
        nc.dram_tensor("sg", [P, G1 * SW], bf16, kind="ExternalInput") if G1 else None
    )
    out = nc.dram_tensor("out", [P, SUBS], f32, kind="ExternalOutput")

    with tile.TileContext(nc) as tc:
        with (
            tc.tile_pool(name="blk", bufs=G2 + 1) as blk_pool,
            tc.tile_pool(name="scr", bufs=2) as scr_pool,
            tc.tile_pool(name="o", bufs=1) as o_pool,
            tc.tile_pool(name="psum", bufs=2, space="PSUM") as psum_pool,
        ):
            ptiles = []
            for g in range(G2):
                t = blk_pool.tile([P, _w(g)], bf16, tag="blk")
                eng = nc.sync if g % 2 == 0 else nc.scalar
                eng.dma_start(out=t[:], in_=pair_in[g][:, :])
                ptiles.append(t)
            sg_tile = None
            if G1:
                sg_tile = blk_pool.tile([P, G1 * SW], bf16, tag="blk")
                eng = nc.sync if G2 % 2 == 0 else nc.scalar
                eng.dma_start(out=sg_tile[:], in_=sg_in[:, :])

            out_sb = o_pool.tile([P, SUBS], f32)

            RHOFF = RB + (DIM if t1x else 0)  # payload base within p[rh]
            for g in range(G2):
                tl = ptiles[g]
                off = RHOFF if g == rh else 0
                col = 2 * g
                if t1x and g == 0:
                    # pair 0: per-subtile psum/DVE; t1 lives in p[rh] prefix
                    ps0 = psum_pool.tile([P, DIM], f32, tag="ps0")
                    for c in range(2):
                        nc.tensor.matmul(
                            out=ps0[:],
                            lhsT=tl[:, 2 * DIM + c * P : 2 * DIM + (c + 1) * P],
                            rhs=tl[:, c * DIM : (c + 1) * DIM],
                            start=(c == 0),
                            stop=(c == 1),
                        )
                    scr0 = scr_pool.tile([P, DIM], bf16, tag="scr")
                    nc.vector.tensor_tensor(
                        out=scr0[:],
                        in0=ps0[:],
                        in1=tl[:, 3 * DIM : 4 * DIM],
                        op=mybir.AluOpType.mult,
                    )
                    nc.vector.reduce_sum(
                        out=out_sb[:, col : col + 1],
                        in_=scr0[:],
                        axis=mybir.AxisListType.X,
                    )
                    ps1 = psum_pool.tile([P, DIM], f32, tag="ps0")
                    for c in range(2):
                        nc.tensor.matmul(
                            out=ps1[0:cap, :],
                            lhsT=ptiles[rh][:, c * cap : (c + 1) * cap],
                            rhs=tl[:, c * DIM : (c + 1) * DIM],
                            start=(c == 0),
                            stop=(c == 1),
                        )
                    scr1 = scr_pool.tile([P, DIM], bf16, tag="scr")
                    nc.vector.tensor_tensor(
                        out=scr1[0:cap, :],
                        in0=ps1[0:cap, :],
                        in1=ptiles[rh][0:cap, RB : RB + DIM],
                        op=mybir.AluOpType.mult,
                    )
                    nc.vector.reduce_sum(
                        out=out_sb[0:cap, col + 1 : col + 2],
                        in_=scr1[0:cap, :],
                        axis=mybir.AxisListType.X,
                    )
                    continue
                ps = psum_pool.tile([P, 2 * DIM], f32, tag="psp")
                # full subtile -> psum cols [0,256)
                for c in range(2):
                    nc.tensor.matmul(
                        out=ps[:, 0:DIM],
                        lhsT=tl[:, off + 2 * DIM + c * P : off + 2 * DIM + (c + 1) * P],
                        rhs=tl[:, off + c * DIM : off + (c + 1) * DIM],
                        start=(c == 0),
                        stop=(c == 1),
                    )
                # remainder subtile -> psum cols [256,512), partitions [0,cap)
                for c in range(2):
                    nc.tensor.matmul(
                        out=ps[0:cap, DIM : 2 * DIM],
                        lhsT=ptiles[rh][
                            :, g * 2 * cap + c * cap : g * 2 * cap + (c + 1) * cap
                        ],
                        rhs=tl[:, off + c * DIM : off + (c + 1) * DIM],
                        start=(c == 0),
                        stop=(c == 1),
                    )
                # one fused multiply over both subtiles (t0|t1 adjacent)
                scr = scr_pool.tile([P, 2 * DIM], bf16, tag="scr")
                nc.vector.tensor_tensor(
                    out=scr[:],
                    in0=ps[:],
                    in1=tl[:, off + 3 * DIM : off + 5 * DIM],
                    op=mybir.AluOpType.mult,
                )
                nc.vector.reduce_sum(
                    out=out_sb[:, col : col + 1],
                    in_=scr[:, 0:DIM],
                    axis=mybir.AxisListType.X,
                )
                nc.vector.reduce_sum(
                    out=out_sb[0:cap, col + 1 : col + 2],
                    in_=scr[0:cap, DIM : 2 * DIM],
                    axis=mybir.AxisListType.X,
                )

            for q in range(G1):
                base = q * SW
                ps = psum_pool.tile([P, DIM], f32, tag="pss")
                for c in range(2):
                    nc.tensor.matmul(
                        out=ps[:],
                        lhsT=sg_tile[
                            :, base + 2 * DIM + c * P : base + 2 * DIM + (c + 1) * P
                        ],
                        rhs=sg_tile[:, base + c * DIM : base + (c + 1) * DIM],
                        start=(c == 0),
                        stop=(c == 1),
                    )
                scr = scr_pool.tile([P, DIM], bf16, tag="scr")
                nc.vector.tensor_tensor(
                    out=scr[:],
                    in0=ps[:],
                    in1=sg_tile[:, base + 3 * DIM : base + 4 * DIM],
                    op=mybir.AluOpType.mult,
                )
                nc.vector.reduce_sum(
                    out=out_sb[:, 2 * G2 + q : 2 * G2 + q + 1],
                    in_=scr[:],
                    axis=mybir.AxisListType.X,
                )

            nc.sync.dma_start(out=out[:, :], in_=out_sb[:])

    nc.compile()
    return nc


def _plan(ids, R):
    """Chunk each relation into <=256-sample segments: pairs (>128 rows)
    and singles (<=128). Round-robin to cores, padded with None to uniform
    (G2, G1). cap = max remainder rows, rounded up to a multiple of 8."""
    pairs, singles = [], []
    for r in range(R):
        idxs = np.nonzero(ids == r)[0]
        for s in range(0, len(idxs), 2 * P):
            seg = idxs[s : s + 2 * P]
            (pairs if len(seg) > P else singles).append((r, seg))
    if not pairs and not singles:
        singles.append((0, np.empty(0, np.int64)))
    G2 = -(-len(pairs) // NCORES) if pairs else 0
    G1 = -(-len(singles) // NCORES) if singles else 0
    cap = 8
    for r, seg in pairs:
        cap = max(cap, len(seg) - P)
    cap = (cap + 7) // 8 * 8
    cores = []
    for k in range(NCORES):
        pk = pairs[k * G2 : (k + 1) * G2] if G2 else []
        sk = singles[k * G1 : (k + 1) * G1] if G1 else []
        pk += [None] * (G2 - len(pk))
        sk += [None] * (G1 - len(sk))
        cores.append((pk, sk))
    return G2, G1, cap, cores


def _core_inputs(head, tail, mstack, pk, sk, G2, G1, cap):
    inp = {}
    RB = G2 * 2 * cap
    rh = 1 if G2 > 1 else 0
    t1x = G2 > 1
    rhoff = RB + (DIM if t1x else 0)
    for g in range(G2):
        off = rhoff if g == rh else 0
        w = PW + (rhoff if g == rh else 0) - (DIM if t1x and g == 0 else 0)
        blk = np.zeros((P, w), BF)
        if pk[g] is not None:
            r, seg = pk[g]
            blk[:, off : off + 2 * DIM] = mstack[r]
            s0, s1 = seg[:P], seg[P:]
            ht = head[s0].astype(BF).T  # [DIM, 128]
            blk[:, off + 2 * DIM : off + 2 * DIM + P] = ht[:P, :]
            blk[:, off + 2 * DIM + P : off + 3 * DIM] = ht[P:, :]
            blk[:, off + 3 * DIM : off + 4 * DIM] = tail[s0].astype(BF)
            nb = len(s1)
            if not (t1x and g == 0):
                blk[:nb, off + 4 * DIM : off + 5 * DIM] = tail[s1].astype(BF)
        inp[f"p{g}"] = blk
    if G2:
        # remainder heads (+ pair 0's t1 when t1x) ride p{rh}'s prefix
        p0 = inp[f"p{rh}"]
        for g in range(G2):
            if pk[g] is None:
                continue
            r, seg = pk[g]
            s1 = seg[P:]
            nb = len(s1)
            if nb:
                ht1 = head[s1].astype(BF).T  # [DIM, nb]
                p0[:, g * 2 * cap : g * 2 * cap + nb] = ht1[:P, :]
                p0[:, g * 2 * cap + cap : g * 2 * cap + cap + nb] = ht1[P:, :]
        if t1x and pk[0] is not None:
            r, seg = pk[0]
            s1 = seg[P:]
            nb = len(s1)
            if nb:
                p0[:nb, RB : RB + DIM] = tail[s1].astype(BF)
    if G1:
        sg = np.zeros((P, G1 * SW), BF)
        for q in range(G1):
            if sk[q] is not None:
                r, seg = sk[q]
                base = q * SW
                sg[:, base : base + 2 * DIM] = mstack[r]
                nb = len(seg)
                ht = head[seg].astype(BF).T
                sg[:, base + 2 * DIM : base + 2 * DIM + nb] = ht[:P, :]
                sg[:, base + 2 * DIM + P : base + 2 * DIM + P + nb] = ht[P:, :]
                sg[:nb, base + 3 * DIM : base + 4 * DIM] = tail[seg].astype(BF)
        inp["sg"] = sg
    return inp


def kernel(head, relation_ids, tail, relation_matrices):
    head = np.ascontiguousarray(np.asarray(head), dtype=np.float32)
    tail = np.ascontiguousarray(np.asarray(tail), dtype=np.float32)
    mats = np.ascontiguousarray(np.asarray(relation_matrices), dtype=np.float32)
    ids = np.asarray(relation_ids).astype(np.int64)
    B, D = head.shape
    R = mats.shape[0]
    assert D == DIM

    G2, G1, cap, cores = _plan(ids, R)
    # [R, P, 2*DIM] bf16: mstack[r, p, c*256+j] = M_r[c*128+p, j]
    mstack = np.ascontiguousarray(
        mats.reshape(R, 2, P, DIM).transpose(0, 2, 1, 3).reshape(R, P, 2 * DIM)
    ).astype(BF)

    in_maps = [
        _core_inputs(head, tail, mstack, pk, sk, G2, G1, cap) for pk, sk in cores
    ]

    key = (G2, G1, cap)
    if key not in _prog_cache:
        _prog_cache[key] = _build(G2, G1, cap)
    nc = _prog_cache[key]

    from concourse.bass_utils import run_bass_kernel_spmd

    kwargs = {}
    if TRACE:
        kwargs = dict(trace=True, trace_cores=list(range(NCORES)))
    try:
        res = run_bass_kernel_spmd(
            nc, in_maps, core_ids=list(range(NCORES)), **kwargs
        )
    except Exception:
        # a previous crashed session can leave the device wedged; one retry
        # after the error has been consumed usually succeeds
        import time as _time

        _time.sleep(2)
        res = run_bass_kernel_spmd(
            nc, in_maps, core_ids=list(range(NCORES)), **kwargs
        )
    global LAST_RESULT
    LAST_RESULT = res

    scores = np.zeros(B, np.float32)
    for k in range(NCORES):
        o = res.results[k]["out"]  # [P, SUBS]
        pk, sk = cores[k]
        for g in range(G2):
            if pk[g] is None:
                continue
            r, seg = pk[g]
            s0, s1 = seg[:P], seg[P:]
            scores[s0] = o[:P, 2 * g]
            scores[s1] = o[: len(s1), 2 * g + 1]
        for q in range(G1):
            if sk[q] is None:
                continue
            r, seg = sk[q]
            scores[seg] = o[: len(seg), 2 * G2 + q]
    return scores
